# revision 7
# baseline (speedup 1.0000x reference)
"""ConvNeXt block kernel for Trainium2 (8 NeuronCores, batch-parallel).

Computes, for x:[B,C,L]:
  p   = depthwise_conv1d(x, dw_w, k=7, pad=3) + dw_b          (per-channel)
  n   = LayerNorm(p.transpose(0,2,1), normalized over [L,C])  (per-batch scalar stats)
  h   = gelu(n @ w1.T + b1)                                   (exact erf gelu)
  y   = h @ w2.T + b2 + x

Sharding: data-parallel over batch, B=16 -> 2 batches per core, no collectives.

Device layout notes:
  - Everything stays in [C, L] layout (C on partitions); LN over (L,C) jointly
    means stats are a single scalar mean/var per batch, so no transpose needed.
  - LN normalization is folded into the GEMM1 epilogue:
      n = p*rs + bcoef  (rs = rsqrt(var+eps), bcoef = -mu*rs, both scalars)
      h = n @ w1.T + b1 = rs*(p @ w1.T) + (bcoef*rowsum(w1) + b1)
    so GEMM1 consumes raw conv output and the activation applies
    scale=rs, bias=bcoef*S1+b1 per h-row. The PE never waits on LN stats.
  - Matmuls run in float32r (fp32 data, full PE rate at free-dim 512).
"""

import sys

if "/opt/trn_rl_repo" not in sys.path:
    sys.path.insert(0, "/opt/trn_rl_repo")

import numpy as np

P = 128
B, C, L, H = 16, 512, 1024, 2048
KW = 7
PAD = 3
CT = C // P          # 4 c-tiles
HT = H // P          # 16 h-tiles
LCW = 512            # l-chunk width (one PSUM bank of fp32)
NLC = L // LCW       # 2 l-chunks
N_CORES = 8
BPC = B // N_CORES   # 2 batches per core
N_ELEMS = float(C * L)
LN_EPS = 1e-5

_prog_cache = {}


def _build_program(sim_act=False):
    # sim_act=True swaps Gelu -> Tanh (CoreSim has no Gelu table); dev only.
    from contextlib import ExitStack

    from concourse import bacc, mybir, tile
    from concourse.alu_op_type import AluOpType

    f32 = mybir.dt.float32
    f32r = mybir.dt.float32r
    AF = mybir.ActivationFunctionType
    AX = mybir.AxisListType

    nc = bacc.Bacc("TRN2", target_bir_lowering=False, debug=False,
                   num_devices=N_CORES)

    x_d = nc.dram_tensor("x", [BPC, C, L], f32, kind="ExternalInput").ap()
    dww_d = nc.dram_tensor("dww", [P, CT * KW], f32, kind="ExternalInput").ap()
    dwb_d = nc.dram_tensor("dwb", [P, CT], f32, kind="ExternalInput").ap()
    w1t_d = nc.dram_tensor("w1t", [C, H], f32r, kind="ExternalInput").ap()
    b1s_d = nc.dram_tensor("b1s", [P, HT], f32, kind="ExternalInput").ap()
    s1s_d = nc.dram_tensor("s1s", [P, HT], f32, kind="ExternalInput").ap()
    w2t_d = nc.dram_tensor("w2t", [H, C], f32r, kind="ExternalInput").ap()
    b2s_d = nc.dram_tensor("b2s", [P, CT], f32, kind="ExternalInput").ap()
    y_d = nc.dram_tensor("y", [BPC, C, L], f32, kind="ExternalOutput").ap()

    with tile.TileContext(nc) as tc, ExitStack() as ctx:
        const = ctx.enter_context(tc.tile_pool(name="const", bufs=1))
        wpool = ctx.enter_context(tc.tile_pool(name="wts", bufs=1))
        xpool = ctx.enter_context(tc.tile_pool(name="xp", bufs=1))
        ppool = ctx.enter_context(tc.tile_pool(name="pp", bufs=1))
        stp = ctx.enter_context(tc.tile_pool(name="stats", bufs=1))
        scr = ctx.enter_context(tc.tile_pool(name="scratch", bufs=2))
        gpool = ctx.enter_context(tc.tile_pool(name="g", bufs=4))
        ypool = ctx.enter_context(tc.tile_pool(name="yo", bufs=4))
        ps_h = ctx.enter_context(tc.tile_pool(name="psh", bufs=2, space="PSUM"))
        ps_y = ctx.enter_context(tc.tile_pool(name="psy", bufs=4, space="PSUM"))
        ps_s = ctx.enter_context(tc.tile_pool(name="pss", bufs=1, space="PSUM"))

        ones = const.tile([P, 1], f32, tag="ones")
        nc.any.memset(ones[:], 1.0)

        dww = const.tile([P, CT * KW], f32, tag="dww")
        nc.sync.dma_start(out=dww[:], in_=dww_d[:])
        dwb = const.tile([P, CT], f32, tag="dwb")
        nc.sync.dma_start(out=dwb[:], in_=dwb_d[:])
        b1s = const.tile([P, HT], f32, tag="b1s")
        nc.sync.dma_start(out=b1s[:], in_=b1s_d[:])
        s1s = const.tile([P, HT], f32, tag="s1s")
        nc.sync.dma_start(out=s1s[:], in_=s1s_d[:])
        b2s = const.tile([P, CT], f32, tag="b2s")
        nc.sync.dma_start(out=b2s[:], in_=b2s_d[:])

        w1 = []
        for ct in range(CT):
            w = wpool.tile([P, H], f32r, tag=f"w1_{ct}")
            nc.sync.dma_start(out=w[:], in_=w1t_d[ct * P:(ct + 1) * P, :])
            w1.append(w)
        w2 = []
        for ht in range(HT):
            w = wpool.tile([P, C], f32r, tag=f"w2_{ht}")
            nc.sync.dma_start(out=w[:], in_=w2t_d[ht * P:(ht + 1) * P, :])
            w2.append(w)

        for b in range(BPC):
            # ---- load x (with zero halo of PAD columns on each side) ----
            xp = []
            for ct in range(CT):
                t = xpool.tile([P, L + 2 * PAD], f32, tag=f"x_{b}_{ct}")
                nc.any.memset(t[:, 0:PAD], 0.0)
                nc.any.memset(t[:, PAD + L:2 * PAD + L], 0.0)
                nc.sync.dma_start(out=t[:, PAD:PAD + L],
                                  in_=x_d[b, ct * P:(ct + 1) * P, :])
                xp.append(t)

            # ---- depthwise conv: p[c,l] = sum_k w[c,k]*x[c,l+k-3] + b[c] ----
            stats = stp.tile([P, 2 * CT], f32, tag=f"st_{b}")
            pt = []
            for ct in range(CT):
                # p is fp32r: every writer of a tile consumed by an fp32r
                # matmul must emit fp32r-rounded values (BIR verifier rule)
                p = ppool.tile([P, L], f32r, tag=f"p_{b}_{ct}")
                pf = p[:].bitcast(f32)  # f32 view for non-matmul consumers
                # center tap first (initializes accumulator, adds dw bias)
                nc.vector.tensor_scalar(
                    p[:], xp[ct][:, PAD:PAD + L],
                    dww[:, ct * KW + PAD:ct * KW + PAD + 1],
                    dwb[:, ct:ct + 1],
                    AluOpType.mult, AluOpType.add)
                taps = [k for k in range(KW) if k != PAD]
                for i, k in enumerate(taps):
                    last = i == len(taps) - 1
                    acc = stats[:, ct:ct + 1] if last else None
                    nc.vector.scalar_tensor_tensor(
                        p[:], xp[ct][:, k:k + L],
                        dww[:, ct * KW + k:ct * KW + k + 1], pf,
                        AluOpType.mult, AluOpType.add, accum_out=acc)
                # sum of squares along the row -> stats[:, CT+ct]
                sq = scr.tile([P, L], f32, tag="sqscr")
                nc.scalar.activation(sq[:], pf, AF.Square,
                                     accum_out=stats[:, CT + ct:CT + ct + 1])
                pt.append(p)

            # ---- LN stats: scalar mean/var over the whole [C,L] plane ----
            sq2 = stp.tile([P, 2], f32, tag=f"sq2_{b}")
            nc.vector.tensor_reduce(sq2[:, 0:1], stats[:, 0:CT], AX.X,
                                    AluOpType.add)
            nc.vector.tensor_reduce(sq2[:, 1:2], stats[:, CT:2 * CT], AX.X,
                                    AluOpType.add)
            pstat = ps_s.tile([1, 2], f32, tag="pstat")
            nc.tensor.matmul(pstat[:], ones[:], sq2[:], start=True, stop=True)

            e = stp.tile([1, 4], f32, tag=f"e_{b}")
            # e[0]=mu, e[1]=E[p^2]
            nc.vector.tensor_scalar(e[:, 0:2], pstat[0:1, :], 1.0 / N_ELEMS,
                                    None, AluOpType.mult)
            # e[2] = var = E[p^2] - mu^2
            nc.vector.scalar_tensor_tensor(e[:, 2:3], e[:, 0:1], -1.0,
                                           e[:, 0:1], AluOpType.mult,
                                           AluOpType.mult)
            # e[3] = var + eps = (E[p^2] + eps) + (-mu^2)
            nc.vector.scalar_tensor_tensor(e[:, 3:4], e[:, 1:2], LN_EPS,
                                           e[:, 2:3], AluOpType.add,
                                           AluOpType.add)
            sd = stp.tile([1, 1], f32, tag=f"sd_{b}")
            nc.scalar.activation(sd[:], e[:, 3:4], AF.Sqrt)
            ab = stp.tile([1, 2], f32, tag=f"ab_{b}")
            nc.vector.reciprocal(ab[:, 0:1], sd[:])           # rs
            nc.vector.scalar_tensor_tensor(ab[:, 1:2], e[:, 0:1], -1.0,
                                           ab[:, 0:1], AluOpType.mult,
                                           AluOpType.mult)    # -mu*rs
            abb = stp.tile([P, 2], f32, tag=f"abb_{b}")
            nc.gpsimd.partition_broadcast(abb[:], ab[:])
            # per-h bias for gelu: bias = bcoef*S1 + b1
            bias16 = stp.tile([P, HT], f32, tag=f"b16_{b}")
            nc.vector.scalar_tensor_tensor(bias16[:], s1s[:], abb[:, 1:2],
                                           b1s[:], AluOpType.mult,
                                           AluOpType.add)

            # ---- GEMM1 -> gelu -> GEMM2 (+bias+residual) per l-chunk ----
            for lc in range(NLC):
                pys = [ps_y.tile([P, LCW], f32, tag="py", name=f"py_{b}_{lc}_{i}")
                       for i in range(CT)]
                for ht in range(HT):
                    ph = ps_h.tile([P, LCW], f32, tag="ph")
                    for ct in range(CT):
                        nc.tensor.matmul(
                            ph[:],
                            w1[ct][:, ht * P:(ht + 1) * P],
                            pt[ct][:, lc * LCW:(lc + 1) * LCW].bitcast(f32r),
                            start=(ct == 0), stop=(ct == CT - 1))
                    g = gpool.tile([P, LCW], f32r, tag="g")
                    act_fn = AF.Tanh if sim_act else AF.Gelu
                    nc.scalar.activation(g[:], ph[:], act_fn,
                                         bias=bias16[:, ht:ht + 1],
                                         scale=abb[:, 0:1])
                    for ct in range(CT):
                        nc.tensor.matmul(
                            pys[ct][:],
                            w2[ht][:, ct * P:(ct + 1) * P],
                            g[:],
                            start=(ht == 0), stop=(ht == HT - 1))
                for ct in range(CT):
                    yt = ypool.tile([P, LCW], f32, tag="yt")
                    nc.vector.scalar_tensor_tensor(
                        yt[:], pys[ct][:], b2s[:, ct:ct + 1],
                        xp[ct][:, PAD + lc * LCW:PAD + (lc + 1) * LCW],
                        AluOpType.add, AluOpType.add)
                    nc.sync.dma_start(
                        out=y_d[b, ct * P:(ct + 1) * P, lc * LCW:(lc + 1) * LCW],
                        in_=yt[:])

    nc.compile()
    return nc


def _get_program():
    if "nc" not in _prog_cache:
        _prog_cache["nc"] = _build_program()
    return _prog_cache["nc"]


def _pack_inputs(x, dw_w, dw_b, w1, b1, w2, b2):
    """Host-side packing into the per-core DRAM tensor layouts."""
    x = np.ascontiguousarray(x, dtype=np.float32)
    dww = np.ascontiguousarray(
        dw_w.reshape(C, KW).reshape(CT, P, KW).transpose(1, 0, 2)
        .reshape(P, CT * KW), dtype=np.float32)
    dwb = np.ascontiguousarray(dw_b.reshape(CT, P).T, dtype=np.float32)
    w1t = np.ascontiguousarray(w1.T, dtype=np.float32)
    b1s = np.ascontiguousarray(b1.reshape(HT, P).T, dtype=np.float32)
    s1s = np.ascontiguousarray(
        w1.astype(np.float32).sum(axis=1).reshape(HT, P).T, dtype=np.float32)
    w2t = np.ascontiguousarray(w2.T, dtype=np.float32)
    b2s = np.ascontiguousarray(b2.reshape(CT, P).T, dtype=np.float32)
    shared = dict(dww=dww, dwb=dwb, w1t=w1t, b1s=b1s, s1s=s1s, w2t=w2t,
                  b2s=b2s)
    in_maps = []
    for c in range(N_CORES):
        m = dict(shared)
        m["x"] = x[c * BPC:(c + 1) * BPC]
        in_maps.append(m)
    return in_maps


def _numpy_fallback(x, dw_w, dw_b, gamma, beta, w1, b1, w2, b2):
    """Pure-host reference path (only used if gamma/beta are non-trivial)."""
    import math
    erf = np.frompyfunc(math.erf, 1, 1)
    x = x.astype(np.float64)
    k = dw_w.reshape(C, KW).astype(np.float64)
    xp = np.pad(x, ((0, 0), (0, 0), (PAD, PAD)))
    p = sum(k[None, :, j:j + 1] * xp[:, :, j:j + L] for j in range(KW))
    p = p + dw_b.astype(np.float64)[None, :, None]
    pt = p.transpose(0, 2, 1)
    mu = pt.mean(axis=(1, 2), keepdims=True)
    var = ((pt - mu) ** 2).mean(axis=(1, 2), keepdims=True)
    n = (pt - mu) / np.sqrt(var + LN_EPS) * gamma.astype(np.float64) \
        + beta.astype(np.float64)
    h = n @ w1.T.astype(np.float64) + b1.astype(np.float64)
    h = 0.5 * h * (1.0 + erf(h / math.sqrt(2.0)).astype(np.float64))
    y = h @ w2.T.astype(np.float64) + b2.astype(np.float64)
    return (y.transpose(0, 2, 1) + x).astype(np.float32)


def kernel(x, dw_w, dw_b, gamma, beta, w1, b1, w2, b2):
    x = np.asarray(x, dtype=np.float32)
    dw_w = np.asarray(dw_w, dtype=np.float32)
    dw_b = np.asarray(dw_b, dtype=np.float32)
    gamma = np.asarray(gamma, dtype=np.float32)
    beta = np.asarray(beta, dtype=np.float32)
    w1 = np.asarray(w1, dtype=np.float32)
    b1 = np.asarray(b1, dtype=np.float32)
    w2 = np.asarray(w2, dtype=np.float32)
    b2 = np.asarray(b2, dtype=np.float32)

    # The device kernel folds LN affine away assuming gamma==1, beta==0
    # (guaranteed by the problem's input spec). Anything else -> host path.
    if not (np.all(gamma == 1.0) and np.all(beta == 0.0)):
        return _numpy_fallback(x, dw_w, dw_b, gamma, beta, w1, b1, w2, b2)

    from concourse.bass_utils import run_bass_kernel_spmd

    nc = _get_program()
    in_maps = _pack_inputs(x, dw_w, dw_b, w1, b1, w2, b2)
    res = run_bass_kernel_spmd(nc, in_maps, list(range(N_CORES)))
    y = np.concatenate([res.results[c]["y"] for c in range(N_CORES)], axis=0)
    return np.ascontiguousarray(y, dtype=np.float32)


# revision 8
# speedup vs baseline: 1.1568x; 1.1568x over previous
"""ConvNeXt block kernel for Trainium2 (8 NeuronCores, batch-parallel).

Computes, for x:[B,C,L]:
  p   = depthwise_conv1d(x, dw_w, k=7, pad=3) + dw_b          (per-channel)
  n   = LayerNorm(p.transpose(0,2,1), normalized over [L,C])  (per-batch scalar stats)
  h   = gelu(n @ w1.T + b1)                                   (exact erf gelu)
  y   = h @ w2.T + b2 + x

Sharding: data-parallel over batch, B=16 -> 2 batches per core, no collectives.

Device layout notes:
  - Everything stays in [C, L] layout (C on partitions); LN over (L,C) jointly
    means stats are a single scalar mean/var per batch, so no transpose needed.
  - LN normalization is folded into the GEMM1 epilogue:
      n = p*rs + bcoef  (rs = rsqrt(var+eps), bcoef = -mu*rs, both scalars)
      h = n @ w1.T + b1 = rs*(p @ w1.T) + (bcoef*rowsum(w1) + b1)
    so GEMM1 consumes raw conv output and the activation applies
    scale=rs, bias=bcoef*S1+b1 per h-row. The PE never waits on LN stats.
  - Matmuls run in float32r (fp32 data, full PE rate at free-dim 512).
"""

import sys

if "/opt/trn_rl_repo" not in sys.path:
    sys.path.insert(0, "/opt/trn_rl_repo")

import numpy as np

P = 128
B, C, L, H = 16, 512, 1024, 2048
KW = 7
PAD = 3
CT = C // P          # 4 c-tiles
HT = H // P          # 16 h-tiles
LCW = 512            # l-chunk width (one PSUM bank of fp32)
NLC = L // LCW       # 2 l-chunks
N_CORES = 8
BPC = B // N_CORES   # 2 batches per core
N_ELEMS = float(C * L)
LN_EPS = 1e-5

_prog_cache = {}


def _build_program(mm_dtype="bf16", sim_act=False):
    """mm_dtype: "f32r" (fp32 data, slow ldweights), "bf16" (fast, ~1e-3 err),
    "mixed" (bf16 weights, f32r activations).
    sim_act=True swaps Gelu -> Tanh (CoreSim has no Gelu table); dev only."""
    from contextlib import ExitStack

    from concourse import bacc, mybir, tile
    from concourse.alu_op_type import AluOpType

    f32 = mybir.dt.float32
    f32r = mybir.dt.float32r
    bf16 = mybir.dt.bfloat16
    w_dt = f32r if mm_dtype == "f32r" else bf16
    m_dt = bf16 if mm_dtype == "bf16" else f32r
    AF = mybir.ActivationFunctionType
    AX = mybir.AxisListType

    nc = bacc.Bacc("TRN2", target_bir_lowering=False, debug=False,
                   num_devices=N_CORES)

    x_d = nc.dram_tensor("x", [BPC, C, L], f32, kind="ExternalInput").ap()
    dww_d = nc.dram_tensor("dww", [P, CT * KW], f32, kind="ExternalInput").ap()
    dwb_d = nc.dram_tensor("dwb", [P, CT], f32, kind="ExternalInput").ap()
    w1t_d = nc.dram_tensor("w1t", [C, H], w_dt, kind="ExternalInput").ap()
    b1s_d = nc.dram_tensor("b1s", [P, HT], f32, kind="ExternalInput").ap()
    s1s_d = nc.dram_tensor("s1s", [P, HT], f32, kind="ExternalInput").ap()
    w2t_d = nc.dram_tensor("w2t", [H, C], w_dt, kind="ExternalInput").ap()
    b2s_d = nc.dram_tensor("b2s", [P, CT], f32, kind="ExternalInput").ap()
    y_d = nc.dram_tensor("y", [BPC, C, L], f32, kind="ExternalOutput").ap()

    with tile.TileContext(nc) as tc, ExitStack() as ctx:
        const = ctx.enter_context(tc.tile_pool(name="const", bufs=1))
        wpool = ctx.enter_context(tc.tile_pool(name="wts", bufs=1))
        xpool = ctx.enter_context(tc.tile_pool(name="xp", bufs=1))
        ppool = ctx.enter_context(tc.tile_pool(name="pp", bufs=1))
        apool = ctx.enter_context(tc.tile_pool(name="acc", bufs=3))
        stp = ctx.enter_context(tc.tile_pool(name="stats", bufs=1))
        scr = ctx.enter_context(tc.tile_pool(name="scratch", bufs=2))
        gpool = ctx.enter_context(tc.tile_pool(name="g", bufs=4))
        ypool = ctx.enter_context(tc.tile_pool(name="yo", bufs=4))
        ps_h = ctx.enter_context(tc.tile_pool(name="psh", bufs=2, space="PSUM"))
        ps_y = ctx.enter_context(tc.tile_pool(name="psy", bufs=4, space="PSUM"))
        ps_s = ctx.enter_context(tc.tile_pool(name="pss", bufs=1, space="PSUM"))

        ones = const.tile([P, 1], f32, tag="ones")
        nc.any.memset(ones[:], 1.0)

        # small constants first, then x(b0) -> w1 -> x(b1) -> w2 so the
        # conv (and then GEMM1) can start as early as possible
        dww = const.tile([P, CT * KW], f32, tag="dww")
        nc.sync.dma_start(out=dww[:], in_=dww_d[:])
        dwb = const.tile([P, CT], f32, tag="dwb")
        nc.sync.dma_start(out=dwb[:], in_=dwb_d[:])
        b1s = const.tile([P, HT], f32, tag="b1s")
        nc.sync.dma_start(out=b1s[:], in_=b1s_d[:])
        s1s = const.tile([P, HT], f32, tag="s1s")
        nc.sync.dma_start(out=s1s[:], in_=s1s_d[:])
        b2s = const.tile([P, CT], f32, tag="b2s")
        nc.sync.dma_start(out=b2s[:], in_=b2s_d[:])

        xp = {}
        for b in range(BPC):
            for ct in range(CT):
                t = xpool.tile([P, L + 2 * PAD], f32, tag=f"x_{b}_{ct}",
                               name=f"x_{b}_{ct}")
                nc.any.memset(t[:, 0:PAD], 0.0)
                nc.any.memset(t[:, PAD + L:2 * PAD + L], 0.0)
                nc.sync.dma_start(out=t[:, PAD:PAD + L],
                                  in_=x_d[b, ct * P:(ct + 1) * P, :])
                xp[b, ct] = t
            if b == 0:
                w1 = []
                for ct in range(CT):
                    w = wpool.tile([P, H], w_dt, tag=f"w1_{ct}")
                    nc.sync.dma_start(out=w[:],
                                      in_=w1t_d[ct * P:(ct + 1) * P, :])
                    w1.append(w)
        w2 = []
        for ht in range(HT):
            w = wpool.tile([P, C], w_dt, tag=f"w2_{ht}")
            nc.sync.dma_start(out=w[:], in_=w2t_d[ht * P:(ht + 1) * P, :])
            w2.append(w)

        for b in range(BPC):
            # ---- depthwise conv, in column halves so GEMM1 of the first
            # l-chunk can start while the second half is still convolving.
            # Taps accumulate in a transient f32 tile; the last tap writes
            # the rounded matmul-dtype tile pb.
            stats = stp.tile([P, 4 * CT], f32, tag=f"st_{b}")
            pb = []
            for ct in range(CT):
                t = ppool.tile([P, L], m_dt, tag=f"p_{b}_{ct}",
                               name=f"p_{b}_{ct}")
                pb.append(t)
            for half in range(NLC):
                o = half * LCW
                for ct in range(CT):
                    acc = apool.tile([P, LCW], f32, tag="acc",
                                     name=f"acc_{b}_{half}_{ct}")
                    xt = xp[b, ct]
                    nc.vector.tensor_scalar(
                        acc[:], xt[:, PAD + o:PAD + o + LCW],
                        dww[:, ct * KW + PAD:ct * KW + PAD + 1],
                        dwb[:, ct:ct + 1],
                        AluOpType.mult, AluOpType.add)
                    taps = [k for k in range(KW) if k != PAD]
                    for i, k in enumerate(taps):
                        last = i == len(taps) - 1
                        out_ap = pb[ct][:, o:o + LCW] if last else acc[:]
                        acc_col = (stats[:, half * CT + ct:half * CT + ct + 1]
                                   if last else None)
                        nc.vector.scalar_tensor_tensor(
                            out_ap, xt[:, k + o:k + o + LCW],
                            dww[:, ct * KW + k:ct * KW + k + 1], acc[:],
                            AluOpType.mult, AluOpType.add, accum_out=acc_col)
                    sq = scr.tile([P, LCW], f32, tag="sqscr",
                                  name=f"sq_{b}_{half}_{ct}")
                    sc_col = 2 * CT + half * CT + ct
                    nc.scalar.activation(sq[:], pb[ct][:, o:o + LCW],
                                         AF.Square,
                                         accum_out=stats[:, sc_col:sc_col + 1])

            # ---- LN stats: scalar mean/var over the whole [C,L] plane ----
            sq2 = stp.tile([P, 2], f32, tag=f"sq2_{b}")
            nc.vector.tensor_reduce(sq2[:, 0:1], stats[:, 0:2 * CT], AX.X,
                                    AluOpType.add)
            nc.vector.tensor_reduce(sq2[:, 1:2], stats[:, 2 * CT:4 * CT], AX.X,
                                    AluOpType.add)
            pstat = ps_s.tile([1, 2], f32, tag="pstat", name=f"pstat_{b}")
            nc.tensor.matmul(pstat[:], ones[:], sq2[:], start=True, stop=True)

            e = stp.tile([1, 4], f32, tag=f"e_{b}")
            # e[0]=mu, e[1]=E[p^2]
            nc.vector.tensor_scalar(e[:, 0:2], pstat[0:1, :], 1.0 / N_ELEMS,
                                    None, AluOpType.mult)
            # e[2] = -mu^2
            nc.vector.scalar_tensor_tensor(e[:, 2:3], e[:, 0:1], -1.0,
                                           e[:, 0:1], AluOpType.mult,
                                           AluOpType.mult)
            # e[3] = var + eps = (E[p^2] + eps) + (-mu^2)
            nc.vector.scalar_tensor_tensor(e[:, 3:4], e[:, 1:2], LN_EPS,
                                           e[:, 2:3], AluOpType.add,
                                           AluOpType.add)
            sd = stp.tile([1, 1], f32, tag=f"sd_{b}")
            nc.scalar.activation(sd[:], e[:, 3:4], AF.Sqrt)
            ab = stp.tile([1, 2], f32, tag=f"ab_{b}")
            nc.vector.reciprocal(ab[:, 0:1], sd[:])           # rs
            nc.vector.scalar_tensor_tensor(ab[:, 1:2], e[:, 0:1], -1.0,
                                           ab[:, 0:1], AluOpType.mult,
                                           AluOpType.mult)    # -mu*rs
            abb = stp.tile([P, 2], f32, tag=f"abb_{b}")
            nc.gpsimd.partition_broadcast(abb[:], ab[:])
            # per-h bias for gelu: bias = bcoef*S1 + b1
            bias16 = stp.tile([P, HT], f32, tag=f"b16_{b}")
            nc.vector.scalar_tensor_tensor(bias16[:], s1s[:], abb[:, 1:2],
                                           b1s[:], AluOpType.mult,
                                           AluOpType.add)

            # ---- GEMM1 -> gelu -> GEMM2 (+bias+residual) per l-chunk ----
            for lc in range(NLC):
                pys = [ps_y.tile([P, LCW], f32, tag="py",
                                 name=f"py_{b}_{lc}_{i}") for i in range(CT)]
                for ht in range(HT):
                    ph = ps_h.tile([P, LCW], f32, tag="ph",
                                   name=f"ph_{b}_{lc}_{ht}")
                    for ct in range(CT):
                        nc.tensor.matmul(
                            ph[:],
                            w1[ct][:, ht * P:(ht + 1) * P],
                            pb[ct][:, lc * LCW:(lc + 1) * LCW],
                            start=(ct == 0), stop=(ct == CT - 1))
                    g = gpool.tile([P, LCW], m_dt, tag="g",
                                   name=f"g_{b}_{lc}_{ht}")
                    act_fn = AF.Tanh if sim_act else AF.Gelu
                    nc.scalar.activation(g[:], ph[:], act_fn,
                                         bias=bias16[:, ht:ht + 1],
                                         scale=abb[:, 0:1])
                    for ct in range(CT):
                        nc.tensor.matmul(
                            pys[ct][:],
                            w2[ht][:, ct * P:(ct + 1) * P],
                            g[:],
                            start=(ht == 0), stop=(ht == HT - 1))
                for ct in range(CT):
                    yt = ypool.tile([P, LCW], f32, tag="yt",
                                    name=f"yt_{b}_{lc}_{ct}")
                    nc.vector.scalar_tensor_tensor(
                        yt[:], pys[ct][:], b2s[:, ct:ct + 1],
                        xp[b, ct][:, PAD + lc * LCW:PAD + (lc + 1) * LCW],
                        AluOpType.add, AluOpType.add)
                    nc.sync.dma_start(
                        out=y_d[b, ct * P:(ct + 1) * P, lc * LCW:(lc + 1) * LCW],
                        in_=yt[:])

    nc.compile()
    return nc


MM_DTYPE = "bf16"


def _get_program():
    key = "nc_" + MM_DTYPE
    if key not in _prog_cache:
        _prog_cache[key] = _build_program(mm_dtype=MM_DTYPE)
    return _prog_cache[key]


def _pack_inputs(x, dw_w, dw_b, w1, b1, w2, b2):
    """Host-side packing into the per-core DRAM tensor layouts."""
    x = np.ascontiguousarray(x, dtype=np.float32)
    dww = np.ascontiguousarray(
        dw_w.reshape(C, KW).reshape(CT, P, KW).transpose(1, 0, 2)
        .reshape(P, CT * KW), dtype=np.float32)
    dwb = np.ascontiguousarray(dw_b.reshape(CT, P).T, dtype=np.float32)
    if MM_DTYPE == "f32r":
        wdt = np.float32
    else:
        import ml_dtypes
        wdt = ml_dtypes.bfloat16
    w1t = np.ascontiguousarray(w1.T.astype(wdt))
    b1s = np.ascontiguousarray(b1.reshape(HT, P).T, dtype=np.float32)
    s1s = np.ascontiguousarray(
        w1.astype(np.float32).sum(axis=1).reshape(HT, P).T, dtype=np.float32)
    w2t = np.ascontiguousarray(w2.T.astype(wdt))
    b2s = np.ascontiguousarray(b2.reshape(CT, P).T, dtype=np.float32)
    shared = dict(dww=dww, dwb=dwb, w1t=w1t, b1s=b1s, s1s=s1s, w2t=w2t,
                  b2s=b2s)
    in_maps = []
    for c in range(N_CORES):
        m = dict(shared)
        m["x"] = x[c * BPC:(c + 1) * BPC]
        in_maps.append(m)
    return in_maps


def _numpy_fallback(x, dw_w, dw_b, gamma, beta, w1, b1, w2, b2):
    """Pure-host reference path (only used if gamma/beta are non-trivial)."""
    import math
    erf = np.frompyfunc(math.erf, 1, 1)
    x = x.astype(np.float64)
    k = dw_w.reshape(C, KW).astype(np.float64)
    xp = np.pad(x, ((0, 0), (0, 0), (PAD, PAD)))
    p = sum(k[None, :, j:j + 1] * xp[:, :, j:j + L] for j in range(KW))
    p = p + dw_b.astype(np.float64)[None, :, None]
    pt = p.transpose(0, 2, 1)
    mu = pt.mean(axis=(1, 2), keepdims=True)
    var = ((pt - mu) ** 2).mean(axis=(1, 2), keepdims=True)
    n = (pt - mu) / np.sqrt(var + LN_EPS) * gamma.astype(np.float64) \
        + beta.astype(np.float64)
    h = n @ w1.T.astype(np.float64) + b1.astype(np.float64)
    h = 0.5 * h * (1.0 + erf(h / math.sqrt(2.0)).astype(np.float64))
    y = h @ w2.T.astype(np.float64) + b2.astype(np.float64)
    return (y.transpose(0, 2, 1) + x).astype(np.float32)


def kernel(x, dw_w, dw_b, gamma, beta, w1, b1, w2, b2):
    x = np.asarray(x, dtype=np.float32)
    dw_w = np.asarray(dw_w, dtype=np.float32)
    dw_b = np.asarray(dw_b, dtype=np.float32)
    gamma = np.asarray(gamma, dtype=np.float32)
    beta = np.asarray(beta, dtype=np.float32)
    w1 = np.asarray(w1, dtype=np.float32)
    b1 = np.asarray(b1, dtype=np.float32)
    w2 = np.asarray(w2, dtype=np.float32)
    b2 = np.asarray(b2, dtype=np.float32)

    # The device kernel folds LN affine away assuming gamma==1, beta==0
    # (guaranteed by the problem's input spec). Anything else -> host path.
    if not (np.all(gamma == 1.0) and np.all(beta == 0.0)):
        return _numpy_fallback(x, dw_w, dw_b, gamma, beta, w1, b1, w2, b2)

    from concourse.bass_utils import run_bass_kernel_spmd

    nc = _get_program()
    in_maps = _pack_inputs(x, dw_w, dw_b, w1, b1, w2, b2)
    res = run_bass_kernel_spmd(nc, in_maps, list(range(N_CORES)))
    y = np.concatenate([res.results[c]["y"] for c in range(N_CORES)], axis=0)
    return np.ascontiguousarray(y, dtype=np.float32)


# revision 11
# speedup vs baseline: 1.2242x; 1.0582x over previous
"""ConvNeXt block kernel for Trainium2 (8 NeuronCores, batch-parallel).

Computes, for x:[B,C,L]:
  p   = depthwise_conv1d(x, dw_w, k=7, pad=3) + dw_b          (per-channel)
  n   = LayerNorm(p.transpose(0,2,1), normalized over [L,C])  (per-batch scalar stats)
  h   = gelu(n @ w1.T + b1)                                   (exact erf gelu)
  y   = h @ w2.T + b2 + x

Sharding: data-parallel over batch, B=16 -> 2 batches per core, no collectives.

Device layout notes:
  - Everything stays in [C, L] layout (C on partitions); LN over (L,C) jointly
    means stats are a single scalar mean/var per batch, so no transpose needed.
  - LN normalization is folded into the GEMM1 epilogue:
      n = p*rs + bcoef  (rs = rsqrt(var+eps), bcoef = -mu*rs, both scalars)
      h = n @ w1.T + b1 = rs*(p @ w1.T) + (bcoef*rowsum(w1) + b1)
    so GEMM1 consumes raw conv output and the activation applies
    scale=rs, bias=bcoef*S1+b1 per h-row. The PE never waits on LN stats.
  - Matmuls run in float32r (fp32 data, full PE rate at free-dim 512).
"""

import sys

if "/opt/trn_rl_repo" not in sys.path:
    sys.path.insert(0, "/opt/trn_rl_repo")

import numpy as np

P = 128
B, C, L, H = 16, 512, 1024, 2048
KW = 7
PAD = 3
CT = C // P          # 4 c-tiles
HT = H // P          # 16 h-tiles
LCW = 512            # l-chunk width (one PSUM bank of fp32)
NLC = L // LCW       # 2 l-chunks
N_CORES = 8
BPC = B // N_CORES   # 2 batches per core
N_ELEMS = float(C * L)
LN_EPS = 1e-5

_prog_cache = {}

ENABLE_LDW_OPT = False


def _patch_walrus_ldw_opt():
    """walrus is invoked with --enable-ldw-opt=false hardcoded; flip it so
    LDWEIGHTS double-buffers and overlaps the running matmul."""
    import concourse.bass_utils as bu

    if getattr(bu, "_ldw_opt_patched", False):
        return
    orig = bu.run_command

    def patched(argv, **kwargs):
        if isinstance(argv, list):
            argv = ["--enable-ldw-opt=true" if a == "--enable-ldw-opt=false"
                    else a for a in argv]
        return orig(argv, **kwargs)

    bu.run_command = patched
    bu._ldw_opt_patched = True


def _build_program(mm_dtype="bf16", sim_act=False):
    """mm_dtype: "f32r" (fp32 data, slow ldweights), "bf16" (fast, ~1e-3 err),
    "mixed" (bf16 weights, f32r activations).
    sim_act=True swaps Gelu -> Tanh (CoreSim has no Gelu table); dev only."""
    from contextlib import ExitStack

    from concourse import bacc, mybir, tile
    from concourse.alu_op_type import AluOpType

    f32 = mybir.dt.float32
    f32r = mybir.dt.float32r
    bf16 = mybir.dt.bfloat16
    w_dt = f32r if mm_dtype == "f32r" else bf16
    m_dt = bf16 if mm_dtype == "bf16" else f32r
    AF = mybir.ActivationFunctionType
    AX = mybir.AxisListType

    nc = bacc.Bacc("TRN2", target_bir_lowering=False, debug=False,
                   num_devices=N_CORES)

    x_d = nc.dram_tensor("x", [BPC, C, L], f32, kind="ExternalInput").ap()
    dww_d = nc.dram_tensor("dww", [P, CT * KW], f32, kind="ExternalInput").ap()
    dwb_d = nc.dram_tensor("dwb", [P, CT], f32, kind="ExternalInput").ap()
    w1t_d = nc.dram_tensor("w1t", [C, H], w_dt, kind="ExternalInput").ap()
    b1s_d = nc.dram_tensor("b1s", [P, HT], f32, kind="ExternalInput").ap()
    s1s_d = nc.dram_tensor("s1s", [P, HT], f32, kind="ExternalInput").ap()
    w2t_d = nc.dram_tensor("w2t", [H, C], w_dt, kind="ExternalInput").ap()
    b2s_d = nc.dram_tensor("b2s", [P, CT], f32, kind="ExternalInput").ap()
    y_d = nc.dram_tensor("y", [BPC, C, L], f32, kind="ExternalOutput").ap()

    with tile.TileContext(nc) as tc, ExitStack() as ctx:
        const = ctx.enter_context(tc.tile_pool(name="const", bufs=1))
        wpool = ctx.enter_context(tc.tile_pool(name="wts", bufs=1))
        xpool = ctx.enter_context(tc.tile_pool(name="xp", bufs=1))
        ppool = ctx.enter_context(tc.tile_pool(name="pp", bufs=1))
        apool = ctx.enter_context(tc.tile_pool(name="acc", bufs=3))
        stp = ctx.enter_context(tc.tile_pool(name="stats", bufs=1))
        scr = ctx.enter_context(tc.tile_pool(name="scratch", bufs=2))
        gpool = ctx.enter_context(tc.tile_pool(name="g", bufs=6))
        ypool = ctx.enter_context(tc.tile_pool(name="yo", bufs=4))
        hpool = ctx.enter_context(tc.tile_pool(name="hpre", bufs=16))
        ps_h = ctx.enter_context(tc.tile_pool(name="psh", bufs=3, space="PSUM"))
        ps_y = ctx.enter_context(tc.tile_pool(name="psy", bufs=4, space="PSUM"))
        ps_s = ctx.enter_context(tc.tile_pool(name="pss", bufs=1, space="PSUM"))

        ones = const.tile([P, 1], f32, tag="ones")
        nc.any.memset(ones[:], 1.0)

        # small constants first, then x(b0) -> w1 -> x(b1) -> w2 so the
        # conv (and then GEMM1) can start as early as possible
        dww = const.tile([P, CT * KW], f32, tag="dww")
        nc.sync.dma_start(out=dww[:], in_=dww_d[:])
        dwb = const.tile([P, CT], f32, tag="dwb")
        nc.sync.dma_start(out=dwb[:], in_=dwb_d[:])
        b1s = const.tile([P, HT], f32, tag="b1s")
        nc.sync.dma_start(out=b1s[:], in_=b1s_d[:])
        s1s = const.tile([P, HT], f32, tag="s1s")
        nc.sync.dma_start(out=s1s[:], in_=s1s_d[:])
        b2s = const.tile([P, CT], f32, tag="b2s")
        nc.sync.dma_start(out=b2s[:], in_=b2s_d[:])

        xp = {}
        for b in range(BPC):
            for ct in range(CT):
                t = xpool.tile([P, L + 2 * PAD], f32, tag=f"x_{b}_{ct}",
                               name=f"x_{b}_{ct}")
                nc.any.memset(t[:, 0:PAD], 0.0)
                nc.any.memset(t[:, PAD + L:2 * PAD + L], 0.0)
                nc.sync.dma_start(out=t[:, PAD:PAD + L],
                                  in_=x_d[b, ct * P:(ct + 1) * P, :])
                xp[b, ct] = t
            if b == 0:
                w1 = []
                for ct in range(CT):
                    w = wpool.tile([P, H], w_dt, tag=f"w1_{ct}")
                    nc.sync.dma_start(out=w[:],
                                      in_=w1t_d[ct * P:(ct + 1) * P, :])
                    w1.append(w)
        w2 = []
        for ht in range(HT):
            w = wpool.tile([P, C], w_dt, tag=f"w2_{ht}")
            nc.sync.dma_start(out=w[:], in_=w2t_d[ht * P:(ht + 1) * P, :])
            w2.append(w)

        for b in range(BPC):
            # ---- depthwise conv, in column halves so GEMM1 of the first
            # l-chunk can start while the second half is still convolving.
            # Taps accumulate in a transient f32 tile; the last tap writes
            # the rounded matmul-dtype tile pb.
            stats = stp.tile([P, 4 * CT], f32, tag=f"st_{b}")
            pb = []
            for ct in range(CT):
                t = ppool.tile([P, L], m_dt, tag=f"p_{b}_{ct}",
                               name=f"p_{b}_{ct}")
                pb.append(t)
            for half in range(NLC):
                o = half * LCW
                for ct in range(CT):
                    acc = apool.tile([P, LCW], f32, tag="acc",
                                     name=f"acc_{b}_{half}_{ct}")
                    xt = xp[b, ct]
                    nc.vector.tensor_scalar(
                        acc[:], xt[:, PAD + o:PAD + o + LCW],
                        dww[:, ct * KW + PAD:ct * KW + PAD + 1],
                        dwb[:, ct:ct + 1],
                        AluOpType.mult, AluOpType.add)
                    taps = [k for k in range(KW) if k != PAD]
                    for i, k in enumerate(taps):
                        last = i == len(taps) - 1
                        out_ap = pb[ct][:, o:o + LCW] if last else acc[:]
                        acc_col = (stats[:, half * CT + ct:half * CT + ct + 1]
                                   if last else None)
                        nc.vector.scalar_tensor_tensor(
                            out_ap, xt[:, k + o:k + o + LCW],
                            dww[:, ct * KW + k:ct * KW + k + 1], acc[:],
                            AluOpType.mult, AluOpType.add, accum_out=acc_col)
                    sq = scr.tile([P, LCW], f32, tag="sqscr",
                                  name=f"sq_{b}_{half}_{ct}")
                    sc_col = 2 * CT + half * CT + ct
                    nc.scalar.activation(sq[:], pb[ct][:, o:o + LCW],
                                         AF.Square,
                                         accum_out=stats[:, sc_col:sc_col + 1])

            # ---- LN stats: scalar mean/var over the whole [C,L] plane ----
            sq2 = stp.tile([P, 2], f32, tag=f"sq2_{b}")
            nc.vector.tensor_reduce(sq2[:, 0:1], stats[:, 0:2 * CT], AX.X,
                                    AluOpType.add)
            nc.vector.tensor_reduce(sq2[:, 1:2], stats[:, 2 * CT:4 * CT], AX.X,
                                    AluOpType.add)
            pstat = ps_s.tile([1, 2], f32, tag="pstat", name=f"pstat_{b}")
            nc.tensor.matmul(pstat[:], ones[:], sq2[:], start=True, stop=True)

            e = stp.tile([1, 4], f32, tag=f"e_{b}")
            # e[0]=mu, e[1]=E[p^2]
            nc.vector.tensor_scalar(e[:, 0:2], pstat[0:1, :], 1.0 / N_ELEMS,
                                    None, AluOpType.mult)
            # e[2] = -mu^2
            nc.vector.scalar_tensor_tensor(e[:, 2:3], e[:, 0:1], -1.0,
                                           e[:, 0:1], AluOpType.mult,
                                           AluOpType.mult)
            # e[3] = var + eps = (E[p^2] + eps) + (-mu^2)
            nc.vector.scalar_tensor_tensor(e[:, 3:4], e[:, 1:2], LN_EPS,
                                           e[:, 2:3], AluOpType.add,
                                           AluOpType.add)
            # rs = rsqrt(var+eps) on DVE (magic seed + 3 Newton steps);
            # ACT Sqrt would force act-table switches on the critical path
            nt = stp.tile([1, 8], f32, tag=f"nt_{b}")
            i32 = mybir.dt.int32
            v = e[:, 3:4]
            nc.vector.tensor_scalar(nt[:, 0:1].bitcast(i32), v.bitcast(i32),
                                    1, None, AluOpType.arith_shift_right)
            nc.vector.tensor_scalar(nt[:, 1:2].bitcast(i32),
                                    nt[:, 0:1].bitcast(i32), -1, 0x5F3759DF,
                                    AluOpType.mult, AluOpType.add)
            nc.vector.tensor_scalar(nt[:, 2:3], v, -0.5, None, AluOpType.mult)
            r, hv = nt[:, 1:2], nt[:, 2:3]
            for it in range(3):
                nc.vector.tensor_tensor(nt[:, 3:4], r, r, AluOpType.mult)
                nc.vector.tensor_tensor(nt[:, 4:5], nt[:, 3:4], hv,
                                        AluOpType.mult)
                nc.vector.tensor_scalar(nt[:, 5:6], nt[:, 4:5], 1.5, None,
                                        AluOpType.add)
                dst = nt[:, 6:7] if it < 2 else None
                if dst is None:
                    ab = stp.tile([1, 2], f32, tag=f"ab_{b}")
                    dst = ab[:, 0:1]
                nc.vector.tensor_tensor(dst, r, nt[:, 5:6], AluOpType.mult)
                r = nt[:, 6:7]
            nc.vector.scalar_tensor_tensor(ab[:, 1:2], e[:, 0:1], -1.0,
                                           ab[:, 0:1], AluOpType.mult,
                                           AluOpType.mult)    # -mu*rs
            abb = stp.tile([P, 2], f32, tag=f"abb_{b}")
            nc.gpsimd.partition_broadcast(abb[:], ab[:])
            # per-h bias for gelu: bias = bcoef*S1 + b1
            bias16 = stp.tile([P, HT], f32, tag=f"b16_{b}")
            nc.vector.scalar_tensor_tensor(bias16[:], s1s[:], abb[:, 1:2],
                                           b1s[:], AluOpType.mult,
                                           AluOpType.add)

            # ---- GEMM1 -> gelu -> GEMM2 (+bias+residual) per l-chunk ----
            act_fn = AF.Tanh if sim_act else AF.Gelu
            for lc in range(NLC):
                pys = [ps_y.tile([P, LCW], f32, tag="py",
                                 name=f"py_{b}_{lc}_{i}") for i in range(CT)]
                # First chunk of batch 0: LN stats are still in flight, so
                # GELU (which needs them) would gate PSUM recycling and
                # stall the PE. Run all of GEMM1 first, evicting the
                # pre-activation tiles to SBUF via ACT copies.
                evict = b == 0 and lc == 0
                hp = {}
                if evict:
                    for ht in range(HT):
                        ph = ps_h.tile([P, LCW], f32, tag="ph",
                                       name=f"ph_{b}_{lc}_{ht}")
                        for ct in range(CT):
                            nc.tensor.matmul(
                                ph[:],
                                w1[ct][:, ht * P:(ht + 1) * P],
                                pb[ct][:, lc * LCW:(lc + 1) * LCW],
                                start=(ct == 0), stop=(ct == CT - 1))
                        hp[ht] = hpool.tile([P, LCW], f32, tag="hp",
                                            name=f"hp_{ht}")
                        nc.scalar.copy(hp[ht][:], ph[:])
                for ht in range(HT):
                    if evict:
                        zin = hp[ht][:]
                    else:
                        ph = ps_h.tile([P, LCW], f32, tag="ph",
                                       name=f"ph_{b}_{lc}_{ht}")
                        for ct in range(CT):
                            nc.tensor.matmul(
                                ph[:],
                                w1[ct][:, ht * P:(ht + 1) * P],
                                pb[ct][:, lc * LCW:(lc + 1) * LCW],
                                start=(ct == 0), stop=(ct == CT - 1))
                        zin = ph[:]
                    g = gpool.tile([P, LCW], m_dt, tag="g",
                                   name=f"g_{b}_{lc}_{ht}")
                    nc.scalar.activation(g[:], zin, act_fn,
                                         bias=bias16[:, ht:ht + 1],
                                         scale=abb[:, 0:1])
                    for ct in range(CT):
                        nc.tensor.matmul(
                            pys[ct][:],
                            w2[ht][:, ct * P:(ct + 1) * P],
                            g[:],
                            start=(ht == 0), stop=(ht == HT - 1))
                for ct in range(CT):
                    yt = ypool.tile([P, LCW], f32, tag="yt",
                                    name=f"yt_{b}_{lc}_{ct}")
                    nc.vector.scalar_tensor_tensor(
                        yt[:], pys[ct][:], b2s[:, ct:ct + 1],
                        xp[b, ct][:, PAD + lc * LCW:PAD + (lc + 1) * LCW],
                        AluOpType.add, AluOpType.add)
                    nc.sync.dma_start(
                        out=y_d[b, ct * P:(ct + 1) * P, lc * LCW:(lc + 1) * LCW],
                        in_=yt[:])

    nc.compile()
    return nc


MM_DTYPE = "bf16"


def _get_program():
    if ENABLE_LDW_OPT:
        _patch_walrus_ldw_opt()
    key = "nc_" + MM_DTYPE
    if key not in _prog_cache:
        _prog_cache[key] = _build_program(mm_dtype=MM_DTYPE)
    return _prog_cache[key]


def _pack_inputs(x, dw_w, dw_b, w1, b1, w2, b2):
    """Host-side packing into the per-core DRAM tensor layouts."""
    x = np.ascontiguousarray(x, dtype=np.float32)
    dww = np.ascontiguousarray(
        dw_w.reshape(C, KW).reshape(CT, P, KW).transpose(1, 0, 2)
        .reshape(P, CT * KW), dtype=np.float32)
    dwb = np.ascontiguousarray(dw_b.reshape(CT, P).T, dtype=np.float32)
    if MM_DTYPE == "f32r":
        wdt = np.float32
    else:
        import ml_dtypes
        wdt = ml_dtypes.bfloat16
    w1t = np.ascontiguousarray(w1.T.astype(wdt))
    b1s = np.ascontiguousarray(b1.reshape(HT, P).T, dtype=np.float32)
    s1s = np.ascontiguousarray(
        w1.astype(np.float32).sum(axis=1).reshape(HT, P).T, dtype=np.float32)
    w2t = np.ascontiguousarray(w2.T.astype(wdt))
    b2s = np.ascontiguousarray(b2.reshape(CT, P).T, dtype=np.float32)
    shared = dict(dww=dww, dwb=dwb, w1t=w1t, b1s=b1s, s1s=s1s, w2t=w2t,
                  b2s=b2s)
    in_maps = []
    for c in range(N_CORES):
        m = dict(shared)
        m["x"] = x[c * BPC:(c + 1) * BPC]
        in_maps.append(m)
    return in_maps


def _numpy_fallback(x, dw_w, dw_b, gamma, beta, w1, b1, w2, b2):
    """Pure-host reference path (only used if gamma/beta are non-trivial)."""
    import math
    erf = np.frompyfunc(math.erf, 1, 1)
    x = x.astype(np.float64)
    k = dw_w.reshape(C, KW).astype(np.float64)
    xp = np.pad(x, ((0, 0), (0, 0), (PAD, PAD)))
    p = sum(k[None, :, j:j + 1] * xp[:, :, j:j + L] for j in range(KW))
    p = p + dw_b.astype(np.float64)[None, :, None]
    pt = p.transpose(0, 2, 1)
    mu = pt.mean(axis=(1, 2), keepdims=True)
    var = ((pt - mu) ** 2).mean(axis=(1, 2), keepdims=True)
    n = (pt - mu) / np.sqrt(var + LN_EPS) * gamma.astype(np.float64) \
        + beta.astype(np.float64)
    h = n @ w1.T.astype(np.float64) + b1.astype(np.float64)
    h = 0.5 * h * (1.0 + erf(h / math.sqrt(2.0)).astype(np.float64))
    y = h @ w2.T.astype(np.float64) + b2.astype(np.float64)
    return (y.transpose(0, 2, 1) + x).astype(np.float32)


def kernel(x, dw_w, dw_b, gamma, beta, w1, b1, w2, b2):
    x = np.asarray(x, dtype=np.float32)
    dw_w = np.asarray(dw_w, dtype=np.float32)
    dw_b = np.asarray(dw_b, dtype=np.float32)
    gamma = np.asarray(gamma, dtype=np.float32)
    beta = np.asarray(beta, dtype=np.float32)
    w1 = np.asarray(w1, dtype=np.float32)
    b1 = np.asarray(b1, dtype=np.float32)
    w2 = np.asarray(w2, dtype=np.float32)
    b2 = np.asarray(b2, dtype=np.float32)

    # The device kernel folds LN affine away assuming gamma==1, beta==0
    # (guaranteed by the problem's input spec). Anything else -> host path.
    if not (np.all(gamma == 1.0) and np.all(beta == 0.0)):
        return _numpy_fallback(x, dw_w, dw_b, gamma, beta, w1, b1, w2, b2)

    from concourse.bass_utils import run_bass_kernel_spmd

    nc = _get_program()
    in_maps = _pack_inputs(x, dw_w, dw_b, w1, b1, w2, b2)
    res = run_bass_kernel_spmd(nc, in_maps, list(range(N_CORES)))
    y = np.concatenate([res.results[c]["y"] for c in range(N_CORES)], axis=0)
    return np.ascontiguousarray(y, dtype=np.float32)


# revision 12
# speedup vs baseline: 1.2880x; 1.0522x over previous
"""ConvNeXt block kernel for Trainium2 (8 NeuronCores, batch-parallel).

Computes, for x:[B,C,L]:
  p   = depthwise_conv1d(x, dw_w, k=7, pad=3) + dw_b          (per-channel)
  n   = LayerNorm(p.transpose(0,2,1), normalized over [L,C])  (per-batch scalar stats)
  h   = gelu(n @ w1.T + b1)                                   (exact erf gelu)
  y   = h @ w2.T + b2 + x

Sharding: data-parallel over batch, B=16 -> 2 batches per core, no collectives.

Device layout notes:
  - Everything stays in [C, L] layout (C on partitions); LN over (L,C) jointly
    means stats are a single scalar mean/var per batch, so no transpose needed.
  - LN normalization is folded into the GEMM1 epilogue:
      n = p*rs + bcoef  (rs = rsqrt(var+eps), bcoef = -mu*rs, both scalars)
      h = n @ w1.T + b1 = rs*(p @ w1.T) + (bcoef*rowsum(w1) + b1)
    so GEMM1 consumes raw conv output and the activation applies
    scale=rs, bias=bcoef*S1+b1 per h-row. The PE never waits on LN stats.
  - Matmuls run in float32r (fp32 data, full PE rate at free-dim 512).
"""

import sys

if "/opt/trn_rl_repo" not in sys.path:
    sys.path.insert(0, "/opt/trn_rl_repo")

import numpy as np

P = 128
B, C, L, H = 16, 512, 1024, 2048
KW = 7
PAD = 3
CT = C // P          # 4 c-tiles
HT = H // P          # 16 h-tiles
LCW = 512            # l-chunk width (one PSUM bank of fp32)
NLC = L // LCW       # 2 l-chunks
N_CORES = 8
BPC = B // N_CORES   # 2 batches per core
N_ELEMS = float(C * L)
LN_EPS = 1e-5

_prog_cache = {}

ENABLE_LDW_OPT = False


def _patch_walrus_ldw_opt():
    """walrus is invoked with --enable-ldw-opt=false hardcoded; flip it so
    LDWEIGHTS double-buffers and overlaps the running matmul."""
    import concourse.bass_utils as bu

    if getattr(bu, "_ldw_opt_patched", False):
        return
    orig = bu.run_command

    def patched(argv, **kwargs):
        if isinstance(argv, list):
            argv = ["--enable-ldw-opt=true" if a == "--enable-ldw-opt=false"
                    else a for a in argv]
        return orig(argv, **kwargs)

    bu.run_command = patched
    bu._ldw_opt_patched = True


def _build_program(mm_dtype="bf16", sim_act=False):
    """mm_dtype: "f32r" (fp32 data, slow ldweights), "bf16" (fast, ~1e-3 err),
    "mixed" (bf16 weights, f32r activations).
    sim_act=True swaps Gelu -> Tanh (CoreSim has no Gelu table); dev only."""
    from contextlib import ExitStack

    from concourse import bacc, mybir, tile
    from concourse.alu_op_type import AluOpType

    f32 = mybir.dt.float32
    f32r = mybir.dt.float32r
    bf16 = mybir.dt.bfloat16
    w_dt = f32r if mm_dtype == "f32r" else bf16
    m_dt = bf16 if mm_dtype == "bf16" else f32r
    AF = mybir.ActivationFunctionType
    AX = mybir.AxisListType

    nc = bacc.Bacc("TRN2", target_bir_lowering=False, debug=False,
                   num_devices=N_CORES)

    x_d = nc.dram_tensor("x", [BPC, C, L], f32, kind="ExternalInput").ap()
    dww_d = nc.dram_tensor("dww", [P, CT * KW], f32, kind="ExternalInput").ap()
    dwb_d = nc.dram_tensor("dwb", [P, CT], f32, kind="ExternalInput").ap()
    w1t_d = nc.dram_tensor("w1t", [C, H], w_dt, kind="ExternalInput").ap()
    b1s_d = nc.dram_tensor("b1s", [P, HT], f32, kind="ExternalInput").ap()
    s1s_d = nc.dram_tensor("s1s", [P, HT], f32, kind="ExternalInput").ap()
    w2t_d = nc.dram_tensor("w2t", [H, C], w_dt, kind="ExternalInput").ap()
    b2s_d = nc.dram_tensor("b2s", [P, CT], f32, kind="ExternalInput").ap()
    y_d = nc.dram_tensor("y", [BPC, C, L], f32, kind="ExternalOutput").ap()

    with tile.TileContext(nc) as tc, ExitStack() as ctx:
        const = ctx.enter_context(tc.tile_pool(name="const", bufs=1))
        wpool = ctx.enter_context(tc.tile_pool(name="wts", bufs=1))
        xpool = ctx.enter_context(tc.tile_pool(name="xp", bufs=1))
        ppool = ctx.enter_context(tc.tile_pool(name="pp", bufs=1))
        apool = ctx.enter_context(tc.tile_pool(name="acc", bufs=3))
        stp = ctx.enter_context(tc.tile_pool(name="stats", bufs=1))
        scr = ctx.enter_context(tc.tile_pool(name="scratch", bufs=2))
        gpool = ctx.enter_context(tc.tile_pool(name="g", bufs=6))
        ypool = ctx.enter_context(tc.tile_pool(name="yo", bufs=4))
        hpool = ctx.enter_context(tc.tile_pool(name="hpre", bufs=16))
        ps_h = ctx.enter_context(tc.tile_pool(name="psh", bufs=3, space="PSUM"))
        ps_y = ctx.enter_context(tc.tile_pool(name="psy", bufs=4, space="PSUM"))
        ps_s = ctx.enter_context(tc.tile_pool(name="pss", bufs=1, space="PSUM"))

        ones = const.tile([P, 1], f32, tag="ones")
        nc.any.memset(ones[:], 1.0)

        # small constants first, then x(b0) -> w1 -> x(b1) -> w2 so the
        # conv (and then GEMM1) can start as early as possible
        dww = const.tile([P, CT * KW], f32, tag="dww")
        nc.sync.dma_start(out=dww[:], in_=dww_d[:])
        dwb = const.tile([P, CT], f32, tag="dwb")
        nc.sync.dma_start(out=dwb[:], in_=dwb_d[:])
        b1s = const.tile([P, HT], f32, tag="b1s")
        nc.sync.dma_start(out=b1s[:], in_=b1s_d[:])
        s1s = const.tile([P, HT], f32, tag="s1s")
        nc.sync.dma_start(out=s1s[:], in_=s1s_d[:])
        b2s = const.tile([P, CT], f32, tag="b2s")
        nc.sync.dma_start(out=b2s[:], in_=b2s_d[:])

        xp = {}
        for b in range(BPC):
            for ct in range(CT):
                t = xpool.tile([P, L + 2 * PAD], f32, tag=f"x_{b}_{ct}",
                               name=f"x_{b}_{ct}")
                nc.any.memset(t[:, 0:PAD], 0.0)
                nc.any.memset(t[:, PAD + L:2 * PAD + L], 0.0)
                nc.sync.dma_start(out=t[:, PAD:PAD + L],
                                  in_=x_d[b, ct * P:(ct + 1) * P, :])
                xp[b, ct] = t
            if b == 0:
                w1 = []
                for ct in range(CT):
                    w = wpool.tile([P, H], w_dt, tag=f"w1_{ct}")
                    nc.sync.dma_start(out=w[:],
                                      in_=w1t_d[ct * P:(ct + 1) * P, :])
                    w1.append(w)
        w2 = []
        for ht in range(HT):
            w = wpool.tile([P, C], w_dt, tag=f"w2_{ht}")
            nc.sync.dma_start(out=w[:], in_=w2t_d[ht * P:(ht + 1) * P, :])
            w2.append(w)

        for b in range(BPC):
            # ---- depthwise conv, in column halves so GEMM1 of the first
            # l-chunk can start while the second half is still convolving.
            # Taps accumulate in a transient f32 tile; the last tap writes
            # the rounded matmul-dtype tile pb.
            stats = stp.tile([P, 4 * CT], f32, tag=f"st_{b}")
            pb = []
            for ct in range(CT):
                t = ppool.tile([P, L], m_dt, tag=f"p_{b}_{ct}",
                               name=f"p_{b}_{ct}")
                pb.append(t)
            for half in range(NLC):
                o = half * LCW
                for ct in range(CT):
                    acc = apool.tile([P, LCW], f32, tag="acc",
                                     name=f"acc_{b}_{half}_{ct}")
                    xt = xp[b, ct]
                    nc.vector.tensor_scalar(
                        acc[:], xt[:, PAD + o:PAD + o + LCW],
                        dww[:, ct * KW + PAD:ct * KW + PAD + 1],
                        dwb[:, ct:ct + 1],
                        AluOpType.mult, AluOpType.add)
                    taps = [k for k in range(KW) if k != PAD]
                    for i, k in enumerate(taps):
                        last = i == len(taps) - 1
                        out_ap = pb[ct][:, o:o + LCW] if last else acc[:]
                        acc_col = (stats[:, half * CT + ct:half * CT + ct + 1]
                                   if last else None)
                        nc.vector.scalar_tensor_tensor(
                            out_ap, xt[:, k + o:k + o + LCW],
                            dww[:, ct * KW + k:ct * KW + k + 1], acc[:],
                            AluOpType.mult, AluOpType.add, accum_out=acc_col)
                    sq = scr.tile([P, LCW], f32, tag="sqscr",
                                  name=f"sq_{b}_{half}_{ct}")
                    sc_col = 2 * CT + half * CT + ct
                    nc.scalar.activation(sq[:], pb[ct][:, o:o + LCW],
                                         AF.Square,
                                         accum_out=stats[:, sc_col:sc_col + 1])

            # ---- LN stats: scalar mean/var over the whole [C,L] plane.
            # high_priority: these tiny serial ops otherwise get scheduled
            # behind the next batch's conv MACs in the static engine order,
            # stretching the stats->first-GELU critical path by >10us.
            hp_ctx = tc.high_priority()
            hp_ctx.__enter__()
            sq2 = stp.tile([P, 2], f32, tag=f"sq2_{b}")
            nc.vector.tensor_reduce(sq2[:, 0:1], stats[:, 0:2 * CT], AX.X,
                                    AluOpType.add)
            nc.vector.tensor_reduce(sq2[:, 1:2], stats[:, 2 * CT:4 * CT], AX.X,
                                    AluOpType.add)
            pstat = ps_s.tile([1, 2], f32, tag="pstat", name=f"pstat_{b}")
            nc.tensor.matmul(pstat[:], ones[:], sq2[:], start=True, stop=True)

            e = stp.tile([1, 4], f32, tag=f"e_{b}")
            # e[0]=mu, e[1]=E[p^2]
            nc.vector.tensor_scalar(e[:, 0:2], pstat[0:1, :], 1.0 / N_ELEMS,
                                    None, AluOpType.mult)
            # e[2] = -mu^2
            nc.vector.scalar_tensor_tensor(e[:, 2:3], e[:, 0:1], -1.0,
                                           e[:, 0:1], AluOpType.mult,
                                           AluOpType.mult)
            # e[3] = var + eps = (E[p^2] + eps) + (-mu^2)
            nc.vector.scalar_tensor_tensor(e[:, 3:4], e[:, 1:2], LN_EPS,
                                           e[:, 2:3], AluOpType.add,
                                           AluOpType.add)
            # rs = rsqrt(var+eps) on DVE (magic seed + 3 Newton steps);
            # ACT Sqrt would force act-table switches on the critical path
            nt = stp.tile([1, 8], f32, tag=f"nt_{b}")
            i32 = mybir.dt.int32
            v = e[:, 3:4]
            nc.vector.tensor_scalar(nt[:, 0:1].bitcast(i32), v.bitcast(i32),
                                    1, None, AluOpType.arith_shift_right)
            nc.vector.tensor_scalar(nt[:, 1:2].bitcast(i32),
                                    nt[:, 0:1].bitcast(i32), -1, 0x5F3759DF,
                                    AluOpType.mult, AluOpType.add)
            nc.vector.tensor_scalar(nt[:, 2:3], v, -0.5, None, AluOpType.mult)
            r, hv = nt[:, 1:2], nt[:, 2:3]
            for it in range(2):
                nc.vector.tensor_tensor(nt[:, 3:4], r, r, AluOpType.mult)
                nc.vector.tensor_tensor(nt[:, 4:5], nt[:, 3:4], hv,
                                        AluOpType.mult)
                nc.vector.tensor_scalar(nt[:, 5:6], nt[:, 4:5], 1.5, None,
                                        AluOpType.add)
                dst = nt[:, 6:7] if it < 1 else None
                if dst is None:
                    ab = stp.tile([1, 2], f32, tag=f"ab_{b}")
                    dst = ab[:, 0:1]
                nc.vector.tensor_tensor(dst, r, nt[:, 5:6], AluOpType.mult)
                r = nt[:, 6:7]
            nc.vector.scalar_tensor_tensor(ab[:, 1:2], e[:, 0:1], -1.0,
                                           ab[:, 0:1], AluOpType.mult,
                                           AluOpType.mult)    # -mu*rs
            abb = stp.tile([P, 2], f32, tag=f"abb_{b}")
            nc.gpsimd.partition_broadcast(abb[:], ab[:])
            # per-h bias for gelu: bias = bcoef*S1 + b1
            bias16 = stp.tile([P, HT], f32, tag=f"b16_{b}")
            nc.vector.scalar_tensor_tensor(bias16[:], s1s[:], abb[:, 1:2],
                                           b1s[:], AluOpType.mult,
                                           AluOpType.add)
            hp_ctx.__exit__(None, None, None)

            # ---- GEMM1 -> gelu -> GEMM2 (+bias+residual) per l-chunk ----
            act_fn = AF.Tanh if sim_act else AF.Gelu
            for lc in range(NLC):
                pys = [ps_y.tile([P, LCW], f32, tag="py",
                                 name=f"py_{b}_{lc}_{i}") for i in range(CT)]
                # First chunk of batch 0: LN stats are still in flight, so
                # GELU (which needs them) would gate PSUM recycling and
                # stall the PE. Run all of GEMM1 first, evicting the
                # pre-activation tiles to SBUF via ACT copies.
                evict = b == 0 and lc == 0
                hp = {}
                if evict:
                    for ht in range(HT):
                        ph = ps_h.tile([P, LCW], f32, tag="ph",
                                       name=f"ph_{b}_{lc}_{ht}")
                        for ct in range(CT):
                            nc.tensor.matmul(
                                ph[:],
                                w1[ct][:, ht * P:(ht + 1) * P],
                                pb[ct][:, lc * LCW:(lc + 1) * LCW],
                                start=(ct == 0), stop=(ct == CT - 1))
                        hp[ht] = hpool.tile([P, LCW], f32, tag="hp",
                                            name=f"hp_{ht}")
                        nc.scalar.copy(hp[ht][:], ph[:])
                for ht in range(HT):
                    if evict:
                        zin = hp[ht][:]
                    else:
                        ph = ps_h.tile([P, LCW], f32, tag="ph",
                                       name=f"ph_{b}_{lc}_{ht}")
                        for ct in range(CT):
                            nc.tensor.matmul(
                                ph[:],
                                w1[ct][:, ht * P:(ht + 1) * P],
                                pb[ct][:, lc * LCW:(lc + 1) * LCW],
                                start=(ct == 0), stop=(ct == CT - 1))
                        zin = ph[:]
                    g = gpool.tile([P, LCW], m_dt, tag="g",
                                   name=f"g_{b}_{lc}_{ht}")
                    nc.scalar.activation(g[:], zin, act_fn,
                                         bias=bias16[:, ht:ht + 1],
                                         scale=abb[:, 0:1])
                    for ct in range(CT):
                        nc.tensor.matmul(
                            pys[ct][:],
                            w2[ht][:, ct * P:(ct + 1) * P],
                            g[:],
                            start=(ht == 0), stop=(ht == HT - 1))
                for ct in range(CT):
                    yt = ypool.tile([P, LCW], f32, tag="yt",
                                    name=f"yt_{b}_{lc}_{ct}")
                    nc.vector.scalar_tensor_tensor(
                        yt[:], pys[ct][:], b2s[:, ct:ct + 1],
                        xp[b, ct][:, PAD + lc * LCW:PAD + (lc + 1) * LCW],
                        AluOpType.add, AluOpType.add)
                    nc.sync.dma_start(
                        out=y_d[b, ct * P:(ct + 1) * P, lc * LCW:(lc + 1) * LCW],
                        in_=yt[:])

    nc.compile()
    return nc


MM_DTYPE = "bf16"


def _get_program():
    if ENABLE_LDW_OPT:
        _patch_walrus_ldw_opt()
    key = "nc_" + MM_DTYPE
    if key not in _prog_cache:
        _prog_cache[key] = _build_program(mm_dtype=MM_DTYPE)
    return _prog_cache[key]


def _pack_inputs(x, dw_w, dw_b, w1, b1, w2, b2):
    """Host-side packing into the per-core DRAM tensor layouts."""
    x = np.ascontiguousarray(x, dtype=np.float32)
    dww = np.ascontiguousarray(
        dw_w.reshape(C, KW).reshape(CT, P, KW).transpose(1, 0, 2)
        .reshape(P, CT * KW), dtype=np.float32)
    dwb = np.ascontiguousarray(dw_b.reshape(CT, P).T, dtype=np.float32)
    if MM_DTYPE == "f32r":
        wdt = np.float32
    else:
        import ml_dtypes
        wdt = ml_dtypes.bfloat16
    w1t = np.ascontiguousarray(w1.T.astype(wdt))
    b1s = np.ascontiguousarray(b1.reshape(HT, P).T, dtype=np.float32)
    s1s = np.ascontiguousarray(
        w1.astype(np.float32).sum(axis=1).reshape(HT, P).T, dtype=np.float32)
    w2t = np.ascontiguousarray(w2.T.astype(wdt))
    b2s = np.ascontiguousarray(b2.reshape(CT, P).T, dtype=np.float32)
    shared = dict(dww=dww, dwb=dwb, w1t=w1t, b1s=b1s, s1s=s1s, w2t=w2t,
                  b2s=b2s)
    in_maps = []
    for c in range(N_CORES):
        m = dict(shared)
        m["x"] = x[c * BPC:(c + 1) * BPC]
        in_maps.append(m)
    return in_maps


def _numpy_fallback(x, dw_w, dw_b, gamma, beta, w1, b1, w2, b2):
    """Pure-host reference path (only used if gamma/beta are non-trivial)."""
    import math
    erf = np.frompyfunc(math.erf, 1, 1)
    x = x.astype(np.float64)
    k = dw_w.reshape(C, KW).astype(np.float64)
    xp = np.pad(x, ((0, 0), (0, 0), (PAD, PAD)))
    p = sum(k[None, :, j:j + 1] * xp[:, :, j:j + L] for j in range(KW))
    p = p + dw_b.astype(np.float64)[None, :, None]
    pt = p.transpose(0, 2, 1)
    mu = pt.mean(axis=(1, 2), keepdims=True)
    var = ((pt - mu) ** 2).mean(axis=(1, 2), keepdims=True)
    n = (pt - mu) / np.sqrt(var + LN_EPS) * gamma.astype(np.float64) \
        + beta.astype(np.float64)
    h = n @ w1.T.astype(np.float64) + b1.astype(np.float64)
    h = 0.5 * h * (1.0 + erf(h / math.sqrt(2.0)).astype(np.float64))
    y = h @ w2.T.astype(np.float64) + b2.astype(np.float64)
    return (y.transpose(0, 2, 1) + x).astype(np.float32)


def kernel(x, dw_w, dw_b, gamma, beta, w1, b1, w2, b2):
    x = np.asarray(x, dtype=np.float32)
    dw_w = np.asarray(dw_w, dtype=np.float32)
    dw_b = np.asarray(dw_b, dtype=np.float32)
    gamma = np.asarray(gamma, dtype=np.float32)
    beta = np.asarray(beta, dtype=np.float32)
    w1 = np.asarray(w1, dtype=np.float32)
    b1 = np.asarray(b1, dtype=np.float32)
    w2 = np.asarray(w2, dtype=np.float32)
    b2 = np.asarray(b2, dtype=np.float32)

    # The device kernel folds LN affine away assuming gamma==1, beta==0
    # (guaranteed by the problem's input spec). Anything else -> host path.
    if not (np.all(gamma == 1.0) and np.all(beta == 0.0)):
        return _numpy_fallback(x, dw_w, dw_b, gamma, beta, w1, b1, w2, b2)

    from concourse.bass_utils import run_bass_kernel_spmd

    nc = _get_program()
    in_maps = _pack_inputs(x, dw_w, dw_b, w1, b1, w2, b2)
    res = run_bass_kernel_spmd(nc, in_maps, list(range(N_CORES)))
    y = np.concatenate([res.results[c]["y"] for c in range(N_CORES)], axis=0)
    return np.ascontiguousarray(y, dtype=np.float32)
